# revision 1
# baseline (speedup 1.0000x reference)
"""Cross-attention Trainium2 kernel (nn_CrossAttention, B=2, L=2048, D=1024,
Dctx=768, 16 heads x 64).

Sharding: 8 cores = 2 (batch) x 4 (head-groups of 4 heads). Each core computes
its batch's Q/K/V projections for its 4 heads, flash-style attention in the
transposed (S^T) domain, and a partial output projection; the host sums the
head-group partials and adds b_o.

All activations live transposed on-chip (xT, ctxT, qT, kT, attnT) so every
matmul contracts over the partition dim with no on-chip transposes. The host
ships x/context pre-transposed. Matmuls run in float32r (fp32 rounded to
11-bit mantissa by the PE, full streaming rate). The softmax denominator is
produced by 32 ones-columns appended per head to the V weights, giving a
replicated d-block in PSUM that feeds a fast-reciprocal normalize on DVE.
"""
import numpy as np

import concourse.bass as bass
import concourse.tile as tile
from concourse import bacc, mybir, bass_utils

F32R = mybir.dt.float32r
F32 = mybir.dt.float32
EXP = mybir.ActivationFunctionType.Exp

# Problem shape (hardcoded per harness contract)
B, LQ, D = 2, 2048, 1024
DCTX = 768
NH, HD = 16, 64
SCALE = 1.0 / 8.0  # 1/sqrt(64)

# Per-core shard: 4 heads (one group), one batch
GH = 4                # heads per core
ONES = 32             # d-replication rows per head
VW = HD + ONES        # 96: per-head width in augmented V
VAW = GH * VW         # 384
KT_Q = D // 128       # 8
KT_C = DCTX // 128    # 6
NLK = LQ // 128       # 16 key tiles
NS = LQ // 512        # 4 query 512-slices
HALF = 1024


def _build():
    nc = bacc.Bacc("TRN2", target_bir_lowering=False, debug=False,
                   enable_asserts=False, num_devices=8)

    xT_d = nc.dram_tensor("xT", (D, LQ), F32R, kind="ExternalInput").ap()
    cT_d = nc.dram_tensor("ctxT", (DCTX, LQ), F32R, kind="ExternalInput").ap()
    wq_d = nc.dram_tensor("wq", (D, 256), F32R, kind="ExternalInput").ap()
    wk_d = nc.dram_tensor("wk", (DCTX, 256), F32R, kind="ExternalInput").ap()
    wv_d = nc.dram_tensor("wv", (DCTX, VAW), F32R, kind="ExternalInput").ap()
    wo_d = nc.dram_tensor("wo", (256, D), F32R, kind="ExternalInput").ap()
    bq_d = nc.dram_tensor("bq", (128, 2), F32, kind="ExternalInput").ap()
    bk_d = nc.dram_tensor("bk", (128, 2), F32, kind="ExternalInput").ap()
    bvb_d = nc.dram_tensor("bvb", (128, VAW), F32, kind="ExternalInput").ap()
    out_d = nc.dram_tensor("outT", (D, LQ), F32, kind="ExternalOutput").ap()
    import os
    dbg = os.environ.get("KDBG") == "1"
    if dbg:
        dpa_d = nc.dram_tensor("dbg_pa", (96, HALF), F32, kind="ExternalOutput").ap()
        drd_d = nc.dram_tensor("dbg_rd", (ONES, HALF), F32, kind="ExternalOutput").ap()
        dst_d = nc.dram_tensor("dbg_st", (128, HALF), F32, kind="ExternalOutput").ap()
        dex_d = nc.dram_tensor("dbg_ex", (128, HALF), F32, kind="ExternalOutput").ap()
        dv_d = nc.dram_tensor("dbg_v", (128, VAW), F32, kind="ExternalOutput").ap()

    with tile.TileContext(nc) as tc:
        with tc.tile_pool(name="w", bufs=1) as wp, \
             tc.tile_pool(name="xt", bufs=10) as xtp, \
             tc.tile_pool(name="ct", bufs=24) as ctp, \
             tc.tile_pool(name="act", bufs=1) as actp, \
             tc.tile_pool(name="expp", bufs=3) as expp, \
             tc.tile_pool(name="rdp", bufs=1) as rdp, \
             tc.tile_pool(name="outp", bufs=3) as outp, \
             tc.tile_pool(name="ps_mm", bufs=2, space="PSUM") as ps_mm, \
             tc.tile_pool(name="ps_s", bufs=2, space="PSUM") as ps_s, \
             tc.tile_pool(name="ps_at", bufs=1, space="PSUM") as ps_at:

            # ---- weights / biases ----
            wq_t = wp.tile([128, KT_Q * 256], F32R, tag="wq")
            nc.sync.dma_start(wq_t[:].rearrange("p (kt m) -> p kt m", m=256),
                              wq_d.rearrange("(kt p) m -> p kt m", p=128))
            wk_t = wp.tile([128, KT_C * 256], F32R, tag="wk")
            nc.sync.dma_start(wk_t[:].rearrange("p (kt m) -> p kt m", m=256),
                              wk_d.rearrange("(kt p) m -> p kt m", p=128))
            wv_t = wp.tile([128, KT_C * VAW], F32R, tag="wv")
            nc.sync.dma_start(wv_t[:].rearrange("p (kt m) -> p kt m", m=384),
                              wv_d.rearrange("(kt p) m -> p kt m", p=128))
            wo_t = wp.tile([128, 2 * D], F32R, tag="wo")
            nc.sync.dma_start(wo_t[:].rearrange("p (p2 m) -> p p2 m", m=1024),
                              wo_d.rearrange("(p2 p) m -> p p2 m", p=128))
            bq_t = wp.tile([128, 2], F32, tag="bq")
            nc.sync.dma_start(bq_t[:], bq_d[:])
            bk_t = wp.tile([128, 2], F32, tag="bk")
            nc.sync.dma_start(bk_t[:], bk_d[:])
            bvb_t = wp.tile([128, VAW], F32, tag="bvb")
            nc.sync.dma_start(bvb_t[:], bvb_d[:])

            # ---- persistent activation tiles ----
            qT = [actp.tile([128, LQ], F32R, tag=f"qT{p}", name=f"qT{p}")
                  for p in range(2)]
            kT = [actp.tile([128, LQ], F32R, tag=f"kT{p}", name=f"kT{p}")
                  for p in range(2)]
            v_t = actp.tile([128, NLK * VAW], F32R, tag="v")
            aT = [actp.tile([128, LQ], F32R, tag=f"aT{p}", name=f"aT{p}")
                  for p in range(2)]

            # ---- K projection (both pairs) + V projection, per 512-slice ----
            ct_tiles = {}
            for s in range(NS):
                for kt in range(KT_C):
                    t = ctp.tile([128, 512], F32R, tag="ct")
                    nc.sync.dma_start(
                        t[:], cT_d[128 * kt:128 * (kt + 1), 512 * s:512 * (s + 1)])
                    ct_tiles[(kt, s)] = t
                for p in range(2):
                    ps = ps_mm.tile([128, 512], F32, tag="mm")
                    for kt in range(KT_C):
                        nc.tensor.matmul(
                            ps[:], wk_t[:, 256 * kt + 128 * p:256 * kt + 128 * (p + 1)],
                            ct_tiles[(kt, s)][:],
                            start=(kt == 0), stop=(kt == KT_C - 1))
                    nc.vector.tensor_scalar_add(
                        kT[p][:, 512 * s:512 * (s + 1)], ps[:], bk_t[:, p:p + 1])

            # ---- Q projection (both pairs), per 512-slice ----
            for s in range(NS):
                xt_tiles = []
                for kt in range(KT_Q):
                    t = xtp.tile([128, 512], F32R, tag="xt")
                    nc.sync.dma_start(
                        t[:], xT_d[128 * kt:128 * (kt + 1), 512 * s:512 * (s + 1)])
                    xt_tiles.append(t)
                for p in range(2):
                    ps = ps_mm.tile([128, 512], F32, tag="mm")
                    for kt in range(KT_Q):
                        nc.tensor.matmul(
                            ps[:], wq_t[:, 256 * kt + 128 * p:256 * kt + 128 * (p + 1)],
                            xt_tiles[kt][:],
                            start=(kt == 0), stop=(kt == KT_Q - 1))
                    nc.vector.tensor_scalar_add(
                        qT[p][:, 512 * s:512 * (s + 1)], ps[:], bq_t[:, p:p + 1])

            def v_chunk(j):
                ps = ps_mm.tile([128, VAW], F32, tag="mm")
                s, jj = j // 4, j % 4
                for kt in range(KT_C):
                    nc.tensor.matmul(
                        ps[:],
                        ct_tiles[(kt, s)][:, 128 * jj:128 * (jj + 1)],
                        wv_t[:, VAW * kt:VAW * (kt + 1)],
                        start=(kt == 0), stop=(kt == KT_C - 1))
                nc.vector.tensor_add(v_t[:, VAW * j:VAW * (j + 1)], ps[:], bvb_t[:])

            # ---- attention per head; V chunks computed inline on first head ----
            for h in range(GH):
                p, m = h // 2, h % 2
                r0 = 64 * m
                for half in range(2):
                    pa = ps_at.tile([96, HALF], F32, tag="attn")
                    for j in range(NLK):
                        if h == 0 and half == 0:
                            v_chunk(j)
                        st = ps_s.tile([128, HALF], F32, tag="s")
                        for n in range(2):
                            nc.tensor.matmul(
                                st[:, 512 * n:512 * (n + 1)],
                                kT[p][r0:r0 + 64, 128 * j:128 * (j + 1)],
                                qT[p][r0:r0 + 64,
                                      HALF * half + 512 * n:HALF * half + 512 * (n + 1)],
                                start=True, stop=True)
                        ex = expp.tile([128, HALF], F32R, tag="expS")
                        if dbg and h == 0 and half == 0 and j == 0:
                            dt0 = expp.tile([128, HALF], F32, tag="expS")
                            nc.vector.tensor_copy(dt0[:], st[:])
                            nc.sync.dma_start(dst_d[:], dt0[:])
                        nc.scalar.activation(ex[:], st[:], EXP, scale=SCALE)
                        if dbg and h == 0 and half == 0 and j == 0:
                            nc.sync.dma_start(dex_d[:], ex[:].bitcast(F32))
                            nc.sync.dma_start(dv_d[:], v_t[:, 0:VAW].bitcast(F32))
                        for n in range(2):
                            nc.tensor.matmul(
                                pa[:, 512 * n:512 * (n + 1)],
                                v_t[:, VAW * j + VW * h:VAW * j + VW * h + VW],
                                ex[:, 512 * n:512 * (n + 1)],
                                start=(j == 0), stop=(j == NLK - 1))
                    # normalize: attnT = attnU * (1/d)
                    rd = rdp.tile([ONES, HALF], F32, tag="rd")
                    rds = rdp.tile([ONES, HALF], F32, tag="rds")
                    dsb = rdp.tile([ONES, HALF], F32, tag="dsb")
                    nc.vector.tensor_copy(dsb[:], pa[64:96, :])
                    nc.vector.reciprocal_approx_accurate(rd[:], dsb[:], rds[:])
                    if dbg and h == 0 and half == 0:
                        dt_ = expp.tile([96, HALF], F32, tag="expS")
                        nc.vector.tensor_copy(dt_[:], pa[:])
                        nc.sync.dma_start(dpa_d[:], dt_[:])
                        nc.sync.dma_start(drd_d[:], rd[:])
                    cols = slice(HALF * half, HALF * (half + 1))
                    nc.vector.tensor_mul(aT[p][r0:r0 + 32, cols], pa[0:32, :], rd[:])
                    nc.vector.tensor_mul(aT[p][r0 + 32:r0 + 64, cols], pa[32:64, :], rd[:])

            # ---- output projection: outT[m-slice, s-slice] ----
            for mo in range(D // 128):
                for s in range(NS):
                    ps = ps_mm.tile([128, 512], F32, tag="mm")
                    for p in range(2):
                        nc.tensor.matmul(
                            ps[:], wo_t[:, D * p + 128 * mo:D * p + 128 * (mo + 1)],
                            aT[p][:, 512 * s:512 * (s + 1)],
                            start=(p == 0), stop=(p == 1))
                    ot = outp.tile([128, 512], F32, tag="out")
                    nc.vector.tensor_copy(ot[:], ps[:])
                    nc.sync.dma_start(
                        out_d[128 * mo:128 * (mo + 1), 512 * s:512 * (s + 1)], ot[:])

    nc.compile()
    return nc


_NC_CACHE = []


def _get_nc():
    if not _NC_CACHE:
        _NC_CACHE.append(_build())
    return _NC_CACHE[0]


def kernel_run(inputs, trace=False, **kw):
    """Run on HW; returns (full_output, BassKernelResults)."""
    x = np.asarray(inputs["x"], np.float32)
    context = np.asarray(inputs["context"], np.float32)
    w_q = np.asarray(inputs["w_q"], np.float32)
    b_q = np.asarray(inputs["b_q"], np.float32)
    w_k = np.asarray(inputs["w_k"], np.float32)
    b_k = np.asarray(inputs["b_k"], np.float32)
    w_v = np.asarray(inputs["w_v"], np.float32)
    b_v = np.asarray(inputs["b_v"], np.float32)
    w_o = np.asarray(inputs["w_o"], np.float32)
    b_o = np.asarray(inputs["b_o"], np.float32)

    maps = []
    f32 = np.float32
    for c in range(8):
        b, g = c // 4, c % 4
        hs = slice(256 * g, 256 * (g + 1))
        wv_aug = np.zeros((DCTX, VAW), f32)
        bvb = np.zeros((128, VAW), f32)
        for h in range(GH):
            cs = slice(256 * g + HD * h, 256 * g + HD * (h + 1))
            wv_aug[:, VW * h:VW * h + HD] = w_v[:, cs]
            bvb[:, VW * h:VW * h + HD] = b_v[None, cs]
            bvb[:, VW * h + HD:VW * (h + 1)] = 1.0
        maps.append({
            "xT": np.ascontiguousarray(x[b].T),
            "ctxT": np.ascontiguousarray(context[b].T),
            "wq": np.ascontiguousarray(w_q[:, hs]),
            "wk": np.ascontiguousarray(w_k[:, hs]),
            "wv": wv_aug,
            "wo": np.ascontiguousarray(w_o[hs, :]),
            "bq": np.ascontiguousarray(b_q[hs].reshape(2, 128).T),
            "bk": np.ascontiguousarray(b_k[hs].reshape(2, 128).T),
            "bvb": bvb,
        })

    nc = _get_nc()
    res = bass_utils.run_bass_kernel_spmd(nc, maps, core_ids=list(range(8)),
                                          trace=trace, **kw)
    out = np.empty((B, LQ, D), np.float32)
    for b in range(B):
        acc = res.results[4 * b]["outT"].astype(np.float32)
        for g in range(1, 4):
            acc = acc + res.results[4 * b + g]["outT"]
        out[b] = acc.T + b_o[None, :]
    return out, res


def kernel(**inputs) -> np.ndarray:
    out, _ = kernel_run(inputs)
    return out



# revision 2
# speedup vs baseline: 1.3745x; 1.3745x over previous
"""Cross-attention Trainium2 kernel (nn_CrossAttention, B=2, L=2048, D=1024,
Dctx=768, 16 heads x 64).

Sharding: 8 cores = 2 (batch) x 4 (head-groups of 4 heads). Each core computes
its batch's Q/K/V projections for its 4 heads, flash-style attention in the
transposed (S^T) domain, and a partial output projection; the host sums the
head-group partials and adds b_o.

All activations live transposed on-chip (xT, ctxT, qT, kT, attnT) so every
matmul contracts over the partition dim with no on-chip transposes. The host
ships x/context pre-transposed and pre-cast to bf16; all matmul operands are
bf16 (full 2.4 GHz PE streaming rate) with fp32 PSUM accumulation. The softmax
denominator is produced by 32 ones-columns appended per head to the V weights,
giving a replicated d-block in PSUM that feeds a fast-reciprocal normalize on
DVE.
"""
import numpy as np
import ml_dtypes

import concourse.bass as bass
import concourse.tile as tile
from concourse import bacc, mybir, bass_utils

BF16 = mybir.dt.bfloat16
F32 = mybir.dt.float32
EXP = mybir.ActivationFunctionType.Exp
NP_BF16 = ml_dtypes.bfloat16

# Problem shape (hardcoded per harness contract)
B, LQ, D = 2, 2048, 1024
DCTX = 768
NH, HD = 16, 64
SCALE = 1.0 / 8.0  # 1/sqrt(64)

# Per-core shard: 4 heads (one group), one batch
GH = 4                # heads per core
ONES = 32             # d-replication rows per head
VW = HD + ONES        # 96: per-head width in augmented V
VAW = GH * VW         # 384
KT_Q = D // 128       # 8
KT_C = DCTX // 128    # 6
NLK = LQ // 128       # 16 key tiles
NS = LQ // 512        # 4 query 512-slices
HALF = 1024


def _build():
    nc = bacc.Bacc("TRN2", target_bir_lowering=False, debug=False,
                   enable_asserts=False, num_devices=8)

    xT_d = nc.dram_tensor("xT", (D, LQ), BF16, kind="ExternalInput").ap()
    cT_d = nc.dram_tensor("ctxT", (DCTX, LQ), BF16, kind="ExternalInput").ap()
    wq_d = nc.dram_tensor("wq", (D, 256), BF16, kind="ExternalInput").ap()
    wk_d = nc.dram_tensor("wk", (DCTX, 256), BF16, kind="ExternalInput").ap()
    wv_d = nc.dram_tensor("wv", (DCTX, VAW), BF16, kind="ExternalInput").ap()
    wo_d = nc.dram_tensor("wo", (256, D), BF16, kind="ExternalInput").ap()
    bq_d = nc.dram_tensor("bq", (128, 2), F32, kind="ExternalInput").ap()
    bk_d = nc.dram_tensor("bk", (128, 2), F32, kind="ExternalInput").ap()
    bvb_d = nc.dram_tensor("bvb", (128, VAW), F32, kind="ExternalInput").ap()
    out_d = nc.dram_tensor("outT", (D, LQ), F32, kind="ExternalOutput").ap()

    with tile.TileContext(nc) as tc:
        with tc.tile_pool(name="w", bufs=1) as wp, \
             tc.tile_pool(name="xt", bufs=10) as xtp, \
             tc.tile_pool(name="ct", bufs=24) as ctp, \
             tc.tile_pool(name="act", bufs=1) as actp, \
             tc.tile_pool(name="expp", bufs=3) as expp, \
             tc.tile_pool(name="rdp", bufs=1) as rdp, \
             tc.tile_pool(name="outp", bufs=3) as outp, \
             tc.tile_pool(name="ps_mm", bufs=2, space="PSUM") as ps_mm, \
             tc.tile_pool(name="ps_s", bufs=2, space="PSUM") as ps_s, \
             tc.tile_pool(name="ps_at", bufs=1, space="PSUM") as ps_at:

            # ---- weights / biases ----
            wq_t = wp.tile([128, KT_Q * 256], BF16, tag="wq")
            nc.sync.dma_start(wq_t[:].rearrange("p (kt m) -> p kt m", m=256),
                              wq_d.rearrange("(kt p) m -> p kt m", p=128))
            wk_t = wp.tile([128, KT_C * 256], BF16, tag="wk")
            nc.sync.dma_start(wk_t[:].rearrange("p (kt m) -> p kt m", m=256),
                              wk_d.rearrange("(kt p) m -> p kt m", p=128))
            wv_t = wp.tile([128, KT_C * VAW], BF16, tag="wv")
            nc.sync.dma_start(wv_t[:].rearrange("p (kt m) -> p kt m", m=384),
                              wv_d.rearrange("(kt p) m -> p kt m", p=128))
            wo_t = wp.tile([128, 2 * D], BF16, tag="wo")
            nc.sync.dma_start(wo_t[:].rearrange("p (p2 m) -> p p2 m", m=1024),
                              wo_d.rearrange("(p2 p) m -> p p2 m", p=128))
            bq_t = wp.tile([128, 2], F32, tag="bq")
            nc.sync.dma_start(bq_t[:], bq_d[:])
            bk_t = wp.tile([128, 2], F32, tag="bk")
            nc.sync.dma_start(bk_t[:], bk_d[:])
            bvb_t = wp.tile([128, VAW], F32, tag="bvb")
            nc.sync.dma_start(bvb_t[:], bvb_d[:])

            # ---- persistent activation tiles ----
            qT = [actp.tile([128, LQ], BF16, tag=f"qT{p}", name=f"qT{p}")
                  for p in range(2)]
            kT = [actp.tile([128, LQ], BF16, tag=f"kT{p}", name=f"kT{p}")
                  for p in range(2)]
            v_t = actp.tile([128, NLK * VAW], BF16, tag="v")
            aT = [actp.tile([128, LQ], BF16, tag=f"aT{p}", name=f"aT{p}")
                  for p in range(2)]

            # ---- K projection (both pairs) + V projection, per 512-slice ----
            ct_tiles = {}
            for s in range(NS):
                for kt in range(KT_C):
                    t = ctp.tile([128, 512], BF16, tag="ct")
                    nc.sync.dma_start(
                        t[:], cT_d[128 * kt:128 * (kt + 1), 512 * s:512 * (s + 1)])
                    ct_tiles[(kt, s)] = t
                for p in range(2):
                    ps = ps_mm.tile([128, 512], F32, tag="mm")
                    for kt in range(KT_C):
                        nc.tensor.matmul(
                            ps[:], wk_t[:, 256 * kt + 128 * p:256 * kt + 128 * (p + 1)],
                            ct_tiles[(kt, s)][:],
                            start=(kt == 0), stop=(kt == KT_C - 1))
                    nc.vector.tensor_scalar_add(
                        kT[p][:, 512 * s:512 * (s + 1)], ps[:], bk_t[:, p:p + 1])

            # ---- Q projection (both pairs), per 512-slice ----
            for s in range(NS):
                xt_tiles = []
                for kt in range(KT_Q):
                    t = xtp.tile([128, 512], BF16, tag="xt")
                    nc.sync.dma_start(
                        t[:], xT_d[128 * kt:128 * (kt + 1), 512 * s:512 * (s + 1)])
                    xt_tiles.append(t)
                for p in range(2):
                    ps = ps_mm.tile([128, 512], F32, tag="mm")
                    for kt in range(KT_Q):
                        nc.tensor.matmul(
                            ps[:], wq_t[:, 256 * kt + 128 * p:256 * kt + 128 * (p + 1)],
                            xt_tiles[kt][:],
                            start=(kt == 0), stop=(kt == KT_Q - 1))
                    nc.vector.tensor_scalar_add(
                        qT[p][:, 512 * s:512 * (s + 1)], ps[:], bq_t[:, p:p + 1])

            def v_chunk(j):
                ps = ps_mm.tile([128, VAW], F32, tag="mm")
                s, jj = j // 4, j % 4
                for kt in range(KT_C):
                    nc.tensor.matmul(
                        ps[:],
                        ct_tiles[(kt, s)][:, 128 * jj:128 * (jj + 1)],
                        wv_t[:, VAW * kt:VAW * (kt + 1)],
                        start=(kt == 0), stop=(kt == KT_C - 1))
                nc.vector.tensor_add(v_t[:, VAW * j:VAW * (j + 1)], ps[:], bvb_t[:])

            # ---- attention per head; V chunks computed inline on first head ----
            for h in range(GH):
                p, m = h // 2, h % 2
                r0 = 64 * m
                for half in range(2):
                    pa = ps_at.tile([96, HALF], F32, tag="attn")
                    for j in range(NLK):
                        if h == 0 and half == 0:
                            v_chunk(j)
                        st = ps_s.tile([128, HALF], F32, tag="s")
                        for n in range(2):
                            nc.tensor.matmul(
                                st[:, 512 * n:512 * (n + 1)],
                                kT[p][r0:r0 + 64, 128 * j:128 * (j + 1)],
                                qT[p][r0:r0 + 64,
                                      HALF * half + 512 * n:HALF * half + 512 * (n + 1)],
                                start=True, stop=True)
                        ex = expp.tile([128, HALF], BF16, tag="expS")
                        nc.scalar.activation(ex[:], st[:], EXP, scale=SCALE)
                        for n in range(2):
                            nc.tensor.matmul(
                                pa[:, 512 * n:512 * (n + 1)],
                                v_t[:, VAW * j + VW * h:VAW * j + VW * h + VW],
                                ex[:, 512 * n:512 * (n + 1)],
                                start=(j == 0), stop=(j == NLK - 1))
                    # normalize: attnT = attnU * (1/d)
                    rd = rdp.tile([ONES, HALF], F32, tag="rd")
                    rds = rdp.tile([ONES, HALF], F32, tag="rds")
                    dsb = rdp.tile([ONES, HALF], F32, tag="dsb")
                    nc.vector.tensor_copy(dsb[:], pa[64:96, :])
                    nc.vector.reciprocal_approx_accurate(rd[:], dsb[:], rds[:])
                    cols = slice(HALF * half, HALF * (half + 1))
                    nc.vector.tensor_mul(aT[p][r0:r0 + 32, cols], pa[0:32, :], rd[:])
                    nc.vector.tensor_mul(aT[p][r0 + 32:r0 + 64, cols], pa[32:64, :], rd[:])

            # ---- output projection: outT[m-slice, s-slice] ----
            for mo in range(D // 128):
                for s in range(NS):
                    ps = ps_mm.tile([128, 512], F32, tag="mm")
                    for p in range(2):
                        nc.tensor.matmul(
                            ps[:], wo_t[:, D * p + 128 * mo:D * p + 128 * (mo + 1)],
                            aT[p][:, 512 * s:512 * (s + 1)],
                            start=(p == 0), stop=(p == 1))
                    ot = outp.tile([128, 512], F32, tag="out")
                    nc.vector.tensor_copy(ot[:], ps[:])
                    nc.sync.dma_start(
                        out_d[128 * mo:128 * (mo + 1), 512 * s:512 * (s + 1)], ot[:])

    nc.compile()
    return nc


_NC_CACHE = []


def _get_nc():
    if not _NC_CACHE:
        _NC_CACHE.append(_build())
    return _NC_CACHE[0]


def kernel_run(inputs, trace=False, **kw):
    """Run on HW; returns (full_output, BassKernelResults)."""
    x = np.asarray(inputs["x"], np.float32)
    context = np.asarray(inputs["context"], np.float32)
    w_q = np.asarray(inputs["w_q"], np.float32)
    b_q = np.asarray(inputs["b_q"], np.float32)
    w_k = np.asarray(inputs["w_k"], np.float32)
    b_k = np.asarray(inputs["b_k"], np.float32)
    w_v = np.asarray(inputs["w_v"], np.float32)
    b_v = np.asarray(inputs["b_v"], np.float32)
    w_o = np.asarray(inputs["w_o"], np.float32)
    b_o = np.asarray(inputs["b_o"], np.float32)

    f32 = np.float32
    xT_bf = [np.ascontiguousarray(x[b].T).astype(NP_BF16) for b in range(B)]
    cT_bf = [np.ascontiguousarray(context[b].T).astype(NP_BF16) for b in range(B)]

    maps = []
    for c in range(8):
        b, g = c // 4, c % 4
        hs = slice(256 * g, 256 * (g + 1))
        wv_aug = np.zeros((DCTX, VAW), f32)
        bvb = np.zeros((128, VAW), f32)
        for h in range(GH):
            cs = slice(256 * g + HD * h, 256 * g + HD * (h + 1))
            wv_aug[:, VW * h:VW * h + HD] = w_v[:, cs]
            bvb[:, VW * h:VW * h + HD] = b_v[None, cs]
            bvb[:, VW * h + HD:VW * (h + 1)] = 1.0
        maps.append({
            "xT": xT_bf[b],
            "ctxT": cT_bf[b],
            "wq": np.ascontiguousarray(w_q[:, hs]).astype(NP_BF16),
            "wk": np.ascontiguousarray(w_k[:, hs]).astype(NP_BF16),
            "wv": wv_aug.astype(NP_BF16),
            "wo": np.ascontiguousarray(w_o[hs, :]).astype(NP_BF16),
            "bq": np.ascontiguousarray(b_q[hs].reshape(2, 128).T),
            "bk": np.ascontiguousarray(b_k[hs].reshape(2, 128).T),
            "bvb": bvb,
        })

    nc = _get_nc()
    res = bass_utils.run_bass_kernel_spmd(nc, maps, core_ids=list(range(8)),
                                          trace=trace, **kw)
    out = np.empty((B, LQ, D), np.float32)
    for b in range(B):
        acc = res.results[4 * b]["outT"].astype(np.float32)
        for g in range(1, 4):
            acc = acc + res.results[4 * b + g]["outT"]
        out[b] = acc.T + b_o[None, :]
    return out, res


def kernel(**inputs) -> np.ndarray:
    out, _ = kernel_run(inputs)
    return out


# revision 10
# speedup vs baseline: 1.4714x; 1.0705x over previous
"""Cross-attention Trainium2 kernel (nn_CrossAttention, B=2, L=2048, D=1024,
Dctx=768, 16 heads x 64).

Sharding: 8 cores = 2 (batch) x 4 (head-groups of 4 heads). Each core computes
its batch's Q/K/V projections for its 4 heads, flash-style attention in the
transposed (S^T) domain, and a partial output projection; the host sums the
head-group partials and adds b_o.

All activations live transposed on-chip (xT, ctxT, qT, kT, attnT) so every
matmul contracts over the partition dim with no on-chip transposes. The host
ships x/context pre-transposed and pre-cast to fp16; all matmul operands are
fp16 (full 2.4 GHz PE streaming rate) with fp32 PSUM accumulation. The softmax
denominator comes from 32 constant ones-rows per head in the V tile (memset
once, never recomputed), giving a replicated d-block in PSUM; normalization
copies the attention PSUM out early (GpSimd) to free the bank, then
reciprocal+scale on DVE. Output partials are fp16, summed on the host.
"""
import numpy as np

import concourse.bass as bass
import concourse.tile as tile
from concourse import bacc, mybir, bass_utils

F16 = mybir.dt.float16
F32 = mybir.dt.float32
EXP = mybir.ActivationFunctionType.Exp
IDENT = mybir.ActivationFunctionType.Identity

# Problem shape (hardcoded per harness contract)
B, LQ, D = 2, 2048, 1024
DCTX = 768
NH, HD = 16, 64
SCALE = 1.0 / 8.0  # 1/sqrt(64)

# Per-core shard: 4 heads (one group), one batch
GH = 4                # heads per core
ONES = 32             # d-replication rows per head
VW = HD + ONES        # 96: per-head width in v_t
VAW = GH * VW         # 384
GD = GH * HD          # 256: real v columns per chunk
KT_Q = D // 128       # 8
KT_C = DCTX // 128    # 6
NLK = LQ // 128       # 16 key tiles
NS = LQ // 512        # 4 query 512-slices
HALF = 1024


def _build():
    nc = bacc.Bacc("TRN2", target_bir_lowering=False, debug=False,
                   enable_asserts=False, num_devices=8)

    xT_d = nc.dram_tensor("xT", (D, LQ), F16, kind="ExternalInput").ap()
    cT_d = nc.dram_tensor("ctxT", (DCTX, LQ), F16, kind="ExternalInput").ap()
    wq_d = nc.dram_tensor("wq", (D, 256), F16, kind="ExternalInput").ap()
    wk_d = nc.dram_tensor("wk", (DCTX, 256), F16, kind="ExternalInput").ap()
    wv_d = nc.dram_tensor("wv", (DCTX, GD), F16, kind="ExternalInput").ap()
    wo_d = nc.dram_tensor("wo", (256, D), F16, kind="ExternalInput").ap()
    bq_d = nc.dram_tensor("bq", (128, 2), F32, kind="ExternalInput").ap()
    bk_d = nc.dram_tensor("bk", (128, 2), F32, kind="ExternalInput").ap()
    bvb_d = nc.dram_tensor("bvb", (128, GD), F32, kind="ExternalInput").ap()
    out_d = nc.dram_tensor("outT", (D, LQ), F16, kind="ExternalOutput").ap()

    with tile.TileContext(nc) as tc:
        with tc.tile_pool(name="w", bufs=1) as wp, \
             tc.tile_pool(name="xt", bufs=10) as xtp, \
             tc.tile_pool(name="ct", bufs=24) as ctp, \
             tc.tile_pool(name="act", bufs=1) as actp, \
             tc.tile_pool(name="expp", bufs=3) as expp, \
             tc.tile_pool(name="scrp", bufs=3) as scrp, \
             tc.tile_pool(name="rdp", bufs=3) as rdp, \
             tc.tile_pool(name="outp", bufs=3) as outp, \
             tc.tile_pool(name="ps_mm", bufs=2, space="PSUM") as ps_mm, \
             tc.tile_pool(name="ps_s", bufs=2, space="PSUM") as ps_s, \
             tc.tile_pool(name="ps_at", bufs=2, space="PSUM") as ps_at:

            # ---- weight/bias tiles (DMAs issued interleaved below) ----
            wq_t = wp.tile([128, KT_Q * 256], F16, tag="wq")
            wk_t = wp.tile([128, KT_C * 256], F16, tag="wk")
            wv_t = wp.tile([128, KT_C * GD], F16, tag="wv")
            wo_t = wp.tile([128, 2 * D], F16, tag="wo")
            bq_t = wp.tile([128, 2], F32, tag="bq")
            bk_t = wp.tile([128, 2], F32, tag="bk")
            bvb_t = wp.tile([128, GD], F32, tag="bvb")

            # K proj needs these first
            nc.sync.dma_start(wk_t[:].rearrange("p (kt m) -> p kt m", m=256),
                              wk_d.rearrange("(kt p) m -> p kt m", p=128))
            nc.sync.dma_start(bk_t[:], bk_d[:])

            # ---- persistent activation tiles ----
            qT = [actp.tile([128, LQ], F16, tag=f"qT{p}", name=f"qT{p}")
                  for p in range(2)]
            kT = [actp.tile([128, LQ], F16, tag=f"kT{p}", name=f"kT{p}")
                  for p in range(2)]
            v_t = actp.tile([128, NLK * VAW], F16, tag="v")
            aT = [actp.tile([128, LQ], F16, tag=f"aT{p}", name=f"aT{p}")
                  for p in range(2)]

            # constant ones-rows of v_t (softmax denominator), set once
            for h in range(GH):
                nc.vector.memset(
                    v_t[:].rearrange("p (j w) -> p j w", w=VAW)
                    [:, :, VW * h + HD:VW * (h + 1)], 1.0)

            # ---- K projection + interleaved weight DMAs, per 512-slice ----
            ct_tiles = {}
            for s in range(NS):
                for kt in range(KT_C):
                    t = ctp.tile([128, 512], F16, tag="ct")
                    nc.sync.dma_start(
                        t[:], cT_d[128 * kt:128 * (kt + 1), 512 * s:512 * (s + 1)])
                    ct_tiles[(kt, s)] = t
                for p in range(2):
                    ps = ps_mm.tile([128, 512], F32, tag="mm")
                    for kt in range(KT_C):
                        nc.tensor.matmul(
                            ps[:], wk_t[:, 256 * kt + 128 * p:256 * kt + 128 * (p + 1)],
                            ct_tiles[(kt, s)][:],
                            start=(kt == 0), stop=(kt == KT_C - 1))
                    nc.scalar.activation(
                        kT[p][:, 512 * s:512 * (s + 1)], ps[:], IDENT,
                        bias=bk_t[:, p:p + 1])
                # stagger the remaining weight loads behind the ct slices
                if s == 0:
                    nc.sync.dma_start(
                        wq_t[:].rearrange("p (kt m) -> p kt m", m=256),
                        wq_d.rearrange("(kt p) m -> p kt m", p=128))
                    nc.sync.dma_start(bq_t[:], bq_d[:])
                elif s == 1:
                    nc.sync.dma_start(
                        wv_t[:].rearrange("p (kt m) -> p kt m", m=GD),
                        wv_d.rearrange("(kt p) m -> p kt m", p=128))
                    nc.sync.dma_start(bvb_t[:], bvb_d[:])
                elif s == 2:
                    nc.sync.dma_start(
                        wo_t[:].rearrange("p (p2 m) -> p p2 m", m=1024),
                        wo_d.rearrange("(p2 p) m -> p p2 m", p=128))

            # ---- Q projection (both pairs), per 512-slice ----
            for s in range(NS):
                xt_tiles = []
                for kt in range(KT_Q):
                    t = xtp.tile([128, 512], F16, tag="xt")
                    nc.sync.dma_start(
                        t[:], xT_d[128 * kt:128 * (kt + 1), 512 * s:512 * (s + 1)])
                    xt_tiles.append(t)
                for p in range(2):
                    ps = ps_mm.tile([128, 512], F32, tag="mm")
                    for kt in range(KT_Q):
                        nc.tensor.matmul(
                            ps[:], wq_t[:, 256 * kt + 128 * p:256 * kt + 128 * (p + 1)],
                            xt_tiles[kt][:],
                            start=(kt == 0), stop=(kt == KT_Q - 1))
                    nc.scalar.activation(
                        qT[p][:, 512 * s:512 * (s + 1)], ps[:], IDENT,
                        bias=bq_t[:, p:p + 1])

            def v_chunk(j):
                ps = ps_mm.tile([128, GD], F32, tag="mm")
                s, jj = j // 4, j % 4
                for kt in range(KT_C):
                    nc.tensor.matmul(
                        ps[:],
                        ct_tiles[(kt, s)][:, 128 * jj:128 * (jj + 1)],
                        wv_t[:, GD * kt:GD * (kt + 1)],
                        start=(kt == 0), stop=(kt == KT_C - 1))
                for h in range(GH):
                    nc.vector.tensor_add(
                        v_t[:, VAW * j + VW * h:VAW * j + VW * h + HD],
                        ps[:, HD * h:HD * (h + 1)],
                        bvb_t[:, HD * h:HD * (h + 1)])

            def out_proj(s):
                for mo in range(D // 128):
                    ps = ps_mm.tile([128, 512], F32, tag="mm")
                    for p in range(2):
                        nc.tensor.matmul(
                            ps[:], wo_t[:, D * p + 128 * mo:D * p + 128 * (mo + 1)],
                            aT[p][:, 512 * s:512 * (s + 1)],
                            start=(p == 0), stop=(p == 1))
                    ot = outp.tile([128, 512], F16, tag="out")
                    nc.vector.tensor_copy(ot[:], ps[:])
                    nc.sync.dma_start(
                        out_d[128 * mo:128 * (mo + 1), 512 * s:512 * (s + 1)], ot[:])

            # ---- attention; V chunks computed inline on first head ----
            for half in range(2):
                for h in range(GH):
                    p, m = h // 2, h % 2
                    r0 = 64 * m
                    pa = [ps_at.tile([96, 512], F32, tag="attn",
                                     name=f"pa{half}_{h}_{n}")
                          for n in range(2)]
                    for j in range(NLK):
                        if h == 0 and half == 0:
                            v_chunk(j)
                        st = ps_s.tile([128, HALF], F32, tag="s")
                        for n in range(2):
                            nc.tensor.matmul(
                                st[:, 512 * n:512 * (n + 1)],
                                kT[p][r0:r0 + 64, 128 * j:128 * (j + 1)],
                                qT[p][r0:r0 + 64,
                                      HALF * half + 512 * n:HALF * half + 512 * (n + 1)],
                                start=True, stop=True)
                        ex = expp.tile([128, HALF], F16, tag="expS")
                        nc.scalar.activation(ex[:], st[:], EXP, scale=SCALE)
                        for n in range(2):
                            nc.tensor.matmul(
                                pa[n][:],
                                v_t[:, VAW * j + VW * h:VAW * j + VW * h + VW],
                                ex[:, 512 * n:512 * (n + 1)],
                                start=(j == 0), stop=(j == NLK - 1))
                    # normalize: copy PSUM out fast (GpSimd) to free the
                    # banks, then attnT = attnU * (1/d) on DVE
                    for n in range(2):
                        # replicate the d-block to 64 partitions (PSUM->SBUF
                        # partition shifts are legal; SBUF->SBUF are not)
                        dsb = rdp.tile([64, 512], F32, tag="dsb")
                        nc.vector.tensor_copy(dsb[0:32, :], pa[n][64:96, :])
                        nc.vector.tensor_copy(dsb[32:64, :], pa[n][64:96, :])
                        scr = scrp.tile([64, 512], F32, tag="scr")
                        nc.vector.tensor_copy(scr[:], pa[n][0:64, :])
                        rd = rdp.tile([64, 512], F32, tag="rd")
                        rds = rdp.tile([64, 512], F32, tag="rds")
                        nc.vector.reciprocal_approx_accurate(rd[:], dsb[:], rds[:])
                        cols = slice(HALF * half + 512 * n,
                                     HALF * half + 512 * (n + 1))
                        nc.vector.tensor_mul(
                            aT[p][r0:r0 + 64, cols], scr[:], rd[:])
                if half == 0:
                    out_proj(0)
                    out_proj(1)
            out_proj(2)
            out_proj(3)

    nc.compile()
    return nc


_NC_CACHE = []


def _get_nc():
    if not _NC_CACHE:
        _NC_CACHE.append(_build())
    return _NC_CACHE[0]


def kernel_run(inputs, trace=False, **kw):
    """Run on HW; returns (full_output, BassKernelResults)."""
    x = np.asarray(inputs["x"], np.float32)
    context = np.asarray(inputs["context"], np.float32)
    w_q = np.asarray(inputs["w_q"], np.float32)
    b_q = np.asarray(inputs["b_q"], np.float32)
    w_k = np.asarray(inputs["w_k"], np.float32)
    b_k = np.asarray(inputs["b_k"], np.float32)
    w_v = np.asarray(inputs["w_v"], np.float32)
    b_v = np.asarray(inputs["b_v"], np.float32)
    w_o = np.asarray(inputs["w_o"], np.float32)
    b_o = np.asarray(inputs["b_o"], np.float32)

    f16 = np.float16
    xT_h = [np.ascontiguousarray(x[b].T).astype(f16) for b in range(B)]
    cT_h = [np.ascontiguousarray(context[b].T).astype(f16) for b in range(B)]

    maps = []
    for c in range(8):
        b, g = c // 4, c % 4
        hs = slice(256 * g, 256 * (g + 1))
        maps.append({
            "xT": xT_h[b],
            "ctxT": cT_h[b],
            "wq": np.ascontiguousarray(w_q[:, hs]).astype(f16),
            "wk": np.ascontiguousarray(w_k[:, hs]).astype(f16),
            "wv": np.ascontiguousarray(w_v[:, hs]).astype(f16),
            "wo": np.ascontiguousarray(w_o[hs, :]).astype(f16),
            "bq": np.ascontiguousarray(b_q[hs].reshape(2, 128).T),
            "bk": np.ascontiguousarray(b_k[hs].reshape(2, 128).T),
            "bvb": np.ascontiguousarray(
                np.broadcast_to(b_v[None, hs], (128, GD)).astype(np.float32)),
        })

    nc = _get_nc()
    res = bass_utils.run_bass_kernel_spmd(nc, maps, core_ids=list(range(8)),
                                          trace=trace, **kw)
    out = np.empty((B, LQ, D), np.float32)
    for b in range(B):
        acc = res.results[4 * b]["outT"].astype(np.float32)
        for g in range(1, 4):
            acc = acc + res.results[4 * b + g]["outT"].astype(np.float32)
        out[b] = acc.T + b_o[None, :]
    return out, res


def kernel(**inputs) -> np.ndarray:
    out, _ = kernel_run(inputs)
    return out
